# revision 25
# baseline (speedup 1.0000x reference)
"""Bass/Tile TRN2 kernel for nn_Attention (soft visual attention).

Math (per batch b):
    U_h   = hidden @ U_w + U_b                      # [A]
    W_s   = img[b] @ W_w + W_b                      # [L, A]
    att   = tanh(W_s + U_h)                         # [L, A]
    e     = att @ v_w  (+ v_b, dropped: softmax-shift-invariant)
    alpha = softmax(e)                              # [L]
    ctx   = alpha @ img[b]                          # [E]

Sharding: data-parallel over batch B=256 across 8 cores (32 each).
Host precomputes biasT = (hidden @ U_w).T + U_b + W_b  (tiny, [A, B]).

Per-core dataflow (groups of 2 batches, 16 groups, 2-stage software
pipeline so PE never waits on the softmax path):
  front(g):
    - gpsimd cast-DMA loads img rows f32->bf16 into SBUF natural tiles
    - PE transposes natural [l, e] tiles into imgT [e, l2] (bf16), with
      the 4 l-pieces chained into one PSUM bank (start/stop chain)
    - DVE/ACT evacuate imgT psum -> SBUF
    - PE: att[a, l2] = sum_e W[e, a] imgT[e, l2]  (bf16, N=392)
    - ACT: attT = tanh(att + biasT[a, b]) psum -> SBUF (bf16)
    - PE: e[1, l2] = sum_a v[a] attT[a, l2]; DVE evac -> e_sb
  back(g):
    - softmax on [2, 196] (reduce_max(negate), exp+accum, recip, scale)
    - PE transpose alpha [2, l] -> alphaT [l, 2] (bf16)
    - PE: ctx[b, n] = sum_lc alphaT[l, b] nat[l, n], batch 0 at psum
      partition 0, batch 1 at partition 32 (PE base-partition rule)
"""

import numpy as np
import ml_dtypes

import concourse.bass as bass
import concourse.tile as tile
from concourse import bacc, mybir
from concourse.bass_utils import run_bass_kernel_spmd
from concourse.masks import make_identity

F32 = mybir.dt.float32
BF16 = mybir.dt.bfloat16
AX = mybir.AxisListType
AF = mybir.ActivationFunctionType

B, L, E, A = 256, 196, 2048, 512
NCORES = 8
BC = B // NCORES  # 32 batches per core
NG = BC // 2  # 16 groups of 2 batches
KC = E // 128  # 16 contraction chunks
AC = A // 128  # 4 a chunks
L2 = 2 * L  # 392: two batches of l packed in the free dim
NE = E // 512  # 4 ctx output chunks

# l-pieces of a 2-batch group: (batch, row offset within batch, dst col, len)
PIECES = [(0, 0, 0, 128), (0, 128, 128, L - 128), (1, 0, L, 128), (1, 128, L + 128, L - 128)]


def _emit(tc):
    nc = tc.nc
    img = nc.dram_tensor("img", [BC * L, E], F32, kind="ExternalInput").ap()
    biasT = nc.dram_tensor("biasT", [A, BC], F32, kind="ExternalInput").ap()
    w = nc.dram_tensor("w", [E, A], BF16, kind="ExternalInput").ap()
    v = nc.dram_tensor("v", [A, 1], BF16, kind="ExternalInput").ap()
    ctx_o = nc.dram_tensor("ctx", [BC, E], F32, kind="ExternalOutput").ap()
    alpha_o = nc.dram_tensor("alpha", [BC, L], F32, kind="ExternalOutput").ap()

    with (
        tc.tile_pool(name="consts", bufs=1) as consts,
        tc.tile_pool(name="natb", bufs=3) as natb_pool,
        tc.tile_pool(name="imgT", bufs=2) as imgT_pool,
        tc.tile_pool(name="attT", bufs=2) as attT_pool,
        tc.tile_pool(name="sm", bufs=3) as sm,
        tc.tile_pool(name="ctxsb", bufs=2) as ctxsb_pool,
        tc.tile_pool(name="ps_tp", bufs=2, space="PSUM") as ps_tp,
        tc.tile_pool(name="ps_att", bufs=2, space="PSUM") as ps_att,
        tc.tile_pool(name="ps_e", bufs=1, space="PSUM") as ps_e,
        tc.tile_pool(name="ps_alT", bufs=1, space="PSUM") as ps_alT,
        tc.tile_pool(name="ps_ctx", bufs=2, space="PSUM") as ps_ctx,
    ):
        ident = consts.tile([128, 128], BF16)
        make_identity(nc, ident)
        w_sb = consts.tile([128, KC, A], BF16)
        nc.sync.dma_start(out=w_sb, in_=w.rearrange("(kc k) a -> k kc a", k=128))
        v_sb = consts.tile([128, AC], BF16)
        nc.sync.dma_start(out=v_sb, in_=v.rearrange("(c k) o -> k (c o)", k=128))
        biasT_sb = consts.tile([128, AC, BC], F32)
        nc.sync.dma_start(out=biasT_sb, in_=biasT.rearrange("(c k) b -> k c b", k=128))

        def loads(g):
            b0 = 2 * g
            nat = []
            for i, (bi, roff, _, ln) in enumerate(PIECES):
                t = natb_pool.tile([ln, E], BF16, tag=f"nat{i}")
                r = (b0 + bi) * L + roff
                if g == 0:
                    # split the cold-start loads so the first transposes
                    # (e-chunks 0-7) can begin at half-load time
                    for h in range(2):
                        nc.gpsimd.dma_start(
                            out=t[:, h * (E // 2) : (h + 1) * (E // 2)],
                            in_=img[r : r + ln, h * (E // 2) : (h + 1) * (E // 2)],
                        )
                else:
                    nc.gpsimd.dma_start(out=t, in_=img[r : r + ln, :])
                nat.append(t)
            return nat

        def transposes(g, nat):
            imgT = imgT_pool.tile([128, KC, L2], BF16, tag="imgT")
            for kc in range(KC):
                tp = ps_tp.tile([128, L2], BF16, tag="tp")
                for i, (bi, roff, coff, ln) in enumerate(PIECES):
                    nc.tensor.matmul(
                        out=tp[:, coff : coff + ln],
                        lhsT=nat[i][:, kc * 128 : (kc + 1) * 128],
                        rhs=ident[:ln, :ln],
                        is_transpose=True,
                        start=(i == 0),
                        stop=(i == 3),
                        skip_group_check=True,
                    )
                # alternate engines so bank evac latency never gates the
                # next transpose chain (ps_tp bufs=2)
                if kc % 2 == 1:
                    nc.scalar.copy(imgT[:, kc, :], tp)
                else:
                    nc.vector.tensor_copy(imgT[:, kc, :], tp)
            return imgT

        def ws_tanh_e(g, imgT):
            b0 = 2 * g
            attT = attT_pool.tile([128, AC, L2], BF16, tag="attT")
            for ac in range(AC):
                att = ps_att.tile([128, L2], F32, tag="att")
                for kc in range(KC):
                    nc.tensor.matmul(
                        out=att,
                        lhsT=w_sb[:, kc, ac * 128 : (ac + 1) * 128],
                        rhs=imgT[:, kc, :],
                        start=(kc == 0),
                        stop=(kc == KC - 1),
                    )
                for h in range(2):
                    nc.scalar.activation(
                        out=attT[:, ac, h * L : (h + 1) * L],
                        in_=att[:, h * L : (h + 1) * L],
                        func=AF.Tanh,
                        bias=biasT_sb[:, ac, b0 + h : b0 + h + 1],
                        scale=1.0,
                    )

            e_ps = ps_e.tile([1, L2], F32, tag="e")
            for ac in range(AC):
                nc.tensor.matmul(
                    out=e_ps,
                    lhsT=v_sb[:, ac : ac + 1],
                    rhs=attT[:, ac, :],
                    start=(ac == 0),
                    stop=(ac == AC - 1),
                )
            e_sb = sm.tile([1, L2], F32, tag="e_sb")
            nc.vector.tensor_copy(e_sb, e_ps)
            return e_sb

        def softmax_smalls(g, e_sb):
            """DVE/ACT/DMA only — no PE instructions."""
            b0 = 2 * g
            es2 = sm.tile([2, L], F32, tag="es2")
            nc.sync.dma_start(out=es2, in_=e_sb)
            nm2 = sm.tile([2, 1], F32, tag="nm2")
            nc.vector.tensor_reduce(
                out=nm2, in_=es2, axis=AX.X, op=mybir.AluOpType.max, negate=True
            )
            u2 = sm.tile([2, L], F32, tag="u2")
            s2 = sm.tile([2, 1], F32, tag="s2")
            nc.scalar.activation(
                out=u2, in_=es2, func=AF.Exp, bias=nm2, scale=1.0, accum_out=s2
            )
            r2 = sm.tile([2, 1], F32, tag="r2")
            nc.vector.reciprocal(r2, s2)
            al2 = sm.tile([2, L], F32, tag="al2")
            nc.vector.tensor_scalar_mul(al2, u2, r2)
            nc.sync.dma_start(out=alpha_o[b0 : b0 + 2, :], in_=al2)
            alb2 = sm.tile([2, L], BF16, tag="alb2")
            nc.vector.tensor_copy(alb2, al2)
            return alb2

        def alpha_T(g, alb2):
            alT_ps = ps_alT.tile([128, 4], BF16, tag="alT")
            nc.tensor.matmul(
                out=alT_ps[:, 0:2],
                lhsT=alb2[:, 0:128],
                rhs=ident[:2, :2],
                is_transpose=True,
                start=True,
                stop=False,
                skip_group_check=True,
            )
            nc.tensor.matmul(
                out=alT_ps[: L - 128, 2:4],
                lhsT=alb2[:, 128:L],
                rhs=ident[:2, :2],
                is_transpose=True,
                start=False,
                stop=True,
                skip_group_check=True,
            )
            alT = sm.tile([128, 4], BF16, tag="alT_sb")
            nc.vector.tensor_copy(alT, alT_ps)
            return alT

        def ctx(g, nat, alT):
            b0 = 2 * g
            # batch 0 -> psum partition 0, batch 1 -> partition 32
            ctx_sb = ctxsb_pool.tile([33, E], F32, tag="ctx_sb")
            for n in range(NE):
                ctxp = ps_ctx.tile([33, 512], F32, tag="ctxp")
                for bi in range(2):
                    for lc in range(2):
                        piece = nat[2 * bi + lc]
                        ln = piece.shape[0]
                        lhsT = alT[:ln, 2 * lc + bi : 2 * lc + bi + 1]
                        nc.tensor.matmul(
                            out=ctxp[32 * bi : 32 * bi + 1, :],
                            lhsT=lhsT,
                            rhs=piece[:, n * 512 : (n + 1) * 512],
                            start=(lc == 0),
                            stop=(lc == 1),
                            skip_group_check=True,
                        )
                if n % 2 == 0:
                    nc.vector.tensor_copy(ctx_sb[:, n * 512 : (n + 1) * 512], ctxp)
                else:
                    nc.scalar.copy(ctx_sb[:, n * 512 : (n + 1) * 512], ctxp)
            nc.sync.dma_start(out=ctx_o[b0 : b0 + 2, :], in_=ctx_sb[::32, :])

        # Fine-grained software pipeline. Per-engine queues run in emission
        # order, so: softmax smalls of group g go to DVE/ACT right before
        # group g+1's transposes occupy PE; the tiny alpha-transpose PE ops
        # come after those transposes (softmax long done -> no PE stall),
        # and ctx(g) runs at the end of g+1's front.
        nat = loads(0)
        imgT = transposes(0, nat)
        e_sb = ws_tanh_e(0, imgT)
        for g in range(NG):
            nat_n = loads(g + 1) if g + 1 < NG else None
            alb2 = softmax_smalls(g, e_sb)
            if nat_n is not None:
                imgT_n = transposes(g + 1, nat_n)
                alT = alpha_T(g, alb2)
                e_sb = ws_tanh_e(g + 1, imgT_n)
            else:
                alT = alpha_T(g, alb2)
            ctx(g, nat, alT)
            nat = nat_n


_NC_CACHE = None


def _build():
    global _NC_CACHE
    if _NC_CACHE is None:
        nc = bacc.Bacc("TRN2", target_bir_lowering=False, debug=False)
        with tile.TileContext(nc) as tc:
            _emit(tc)
        nc.compile()
        _NC_CACHE = nc
    return _NC_CACHE


def kernel(img_features, hidden_state, U_w, U_b, W_w, W_b, v_w, v_b):
    img_features = np.asarray(img_features, dtype=np.float32)
    hidden_state = np.asarray(hidden_state, dtype=np.float32)
    U_w = np.asarray(U_w, dtype=np.float32)
    U_b = np.asarray(U_b, dtype=np.float32)
    W_w = np.asarray(W_w, dtype=np.float32)
    W_b = np.asarray(W_b, dtype=np.float32)
    v_w = np.asarray(v_w, dtype=np.float32)

    # host precompute: combined per-(a, b) tanh bias, [A, B]
    biasT = (hidden_state @ U_w).T + (U_b + W_b)[:, None]
    biasT = np.ascontiguousarray(biasT, dtype=np.float32)
    w_bf = W_w.astype(ml_dtypes.bfloat16)
    v_bf = v_w.astype(ml_dtypes.bfloat16)

    nc = _build()
    in_maps = []
    for c in range(NCORES):
        sl = slice(c * BC, (c + 1) * BC)
        in_maps.append(
            {
                "img": np.ascontiguousarray(
                    img_features[sl].reshape(BC * L, E), dtype=np.float32
                ),
                "biasT": np.ascontiguousarray(biasT[:, sl]),
                "w": w_bf,
                "v": v_bf,
            }
        )
    res = run_bass_kernel_spmd(nc, in_maps, list(range(NCORES)))
    ctx = np.concatenate([res.results[c]["ctx"] for c in range(NCORES)], axis=0)
    alpha = np.concatenate([res.results[c]["alpha"] for c in range(NCORES)], axis=0)
    return ctx, alpha


# revision 26
# speedup vs baseline: 1.0280x; 1.0280x over previous
"""Bass/Tile TRN2 kernel for nn_Attention (soft visual attention).

Math (per batch b):
    U_h   = hidden @ U_w + U_b                      # [A]
    W_s   = img[b] @ W_w + W_b                      # [L, A]
    att   = tanh(W_s + U_h)                         # [L, A]
    e     = att @ v_w  (+ v_b, dropped: softmax-shift-invariant)
    alpha = softmax(e)                              # [L]
    ctx   = alpha @ img[b]                          # [E]

Sharding: data-parallel over batch B=256 across 8 cores (32 each).
Host precomputes biasT = (hidden @ U_w).T + U_b + W_b  (tiny, [A, B]).

Per-core dataflow (groups of 2 batches, 16 groups, 2-stage software
pipeline so PE never waits on the softmax path):
  front(g):
    - gpsimd cast-DMA loads img rows f32->bf16 into SBUF natural tiles
    - PE transposes natural [l, e] tiles into imgT [e, l2] (bf16), with
      the 4 l-pieces chained into one PSUM bank (start/stop chain)
    - DVE/ACT evacuate imgT psum -> SBUF
    - PE: att[a, l2] = sum_e W[e, a] imgT[e, l2]  (bf16, N=392)
    - ACT: attT = tanh(att + biasT[a, b]) psum -> SBUF (bf16)
    - PE: e[1, l2] = sum_a v[a] attT[a, l2]; DVE evac -> e_sb
  back(g):
    - softmax on [2, 196] (reduce_max(negate), exp+accum, recip, scale)
    - PE transpose alpha [2, l] -> alphaT [l, 2] (bf16)
    - PE: ctx[b, n] = sum_lc alphaT[l, b] nat[l, n], batch 0 at psum
      partition 0, batch 1 at partition 32 (PE base-partition rule)
"""

import numpy as np
import ml_dtypes

import concourse.bass as bass
import concourse.tile as tile
from concourse import bacc, mybir
from concourse.bass_utils import run_bass_kernel_spmd
from concourse.masks import make_identity

F32 = mybir.dt.float32
BF16 = mybir.dt.bfloat16
AX = mybir.AxisListType
AF = mybir.ActivationFunctionType

B, L, E, A = 256, 196, 2048, 512
NCORES = 8
BC = B // NCORES  # 32 batches per core
NG = BC // 2  # 16 groups of 2 batches
KC = E // 128  # 16 contraction chunks
AC = A // 128  # 4 a chunks
L2 = 2 * L  # 392: two batches of l packed in the free dim
NE = E // 512  # 4 ctx output chunks

# l-pieces of a 2-batch group: (batch, row offset within batch, dst col, len)
PIECES = [(0, 0, 0, 128), (0, 128, 128, L - 128), (1, 0, L, 128), (1, 128, L + 128, L - 128)]


def _emit(tc):
    nc = tc.nc
    img = nc.dram_tensor("img", [BC * L, E], F32, kind="ExternalInput").ap()
    biasT = nc.dram_tensor("biasT", [A, BC], F32, kind="ExternalInput").ap()
    w = nc.dram_tensor("w", [E, A], BF16, kind="ExternalInput").ap()
    v = nc.dram_tensor("v", [A, 1], BF16, kind="ExternalInput").ap()
    ctx_o = nc.dram_tensor("ctx", [BC, E], F32, kind="ExternalOutput").ap()
    alpha_o = nc.dram_tensor("alpha", [BC, L], F32, kind="ExternalOutput").ap()

    with (
        tc.tile_pool(name="consts", bufs=1) as consts,
        tc.tile_pool(name="natb", bufs=3) as natb_pool,
        tc.tile_pool(name="imgT", bufs=2) as imgT_pool,
        tc.tile_pool(name="attT", bufs=2) as attT_pool,
        tc.tile_pool(name="sm", bufs=3) as sm,
        tc.tile_pool(name="ctxsb", bufs=2) as ctxsb_pool,
        tc.tile_pool(name="ps_tp", bufs=2, space="PSUM") as ps_tp,
        tc.tile_pool(name="ps_att", bufs=2, space="PSUM") as ps_att,
        tc.tile_pool(name="ps_e", bufs=1, space="PSUM") as ps_e,
        tc.tile_pool(name="ps_alT", bufs=1, space="PSUM") as ps_alT,
        tc.tile_pool(name="ps_ctx", bufs=2, space="PSUM") as ps_ctx,
    ):
        ident = consts.tile([128, 128], BF16)
        make_identity(nc, ident)
        w_sb = consts.tile([128, KC, A], BF16)
        nc.sync.dma_start(out=w_sb, in_=w.rearrange("(kc k) a -> k kc a", k=128))
        v_sb = consts.tile([128, AC], BF16)
        nc.sync.dma_start(out=v_sb, in_=v.rearrange("(c k) o -> k (c o)", k=128))
        biasT_sb = consts.tile([128, AC, BC], F32)
        nc.sync.dma_start(out=biasT_sb, in_=biasT.rearrange("(c k) b -> k c b", k=128))

        def loads(g):
            b0 = 2 * g
            nat = []
            for i, (bi, roff, _, ln) in enumerate(PIECES):
                t = natb_pool.tile([ln, E], BF16, tag=f"nat{i}")
                r = (b0 + bi) * L + roff
                if g == 0:
                    # split the cold-start loads so the first transposes
                    # (e-chunks 0-7) can begin at half-load time
                    for h in range(2):
                        nc.gpsimd.dma_start(
                            out=t[:, h * (E // 2) : (h + 1) * (E // 2)],
                            in_=img[r : r + ln, h * (E // 2) : (h + 1) * (E // 2)],
                        )
                else:
                    nc.gpsimd.dma_start(out=t, in_=img[r : r + ln, :])
                nat.append(t)
            return nat

        def transposes(g, nat):
            imgT = imgT_pool.tile([128, KC, L2], BF16, tag="imgT")
            for kcp in range(KC // 2):
                # two e-chunks share one psum bank (2x392 bf16 = 1568B)
                tp = ps_tp.tile([128, 2, L2], BF16, tag="tp")
                for sub in range(2):
                    kc = 2 * kcp + sub
                    for i, (bi, roff, coff, ln) in enumerate(PIECES):
                        nc.tensor.matmul(
                            out=tp[:, sub, coff : coff + ln],
                            lhsT=nat[i][:, kc * 128 : (kc + 1) * 128],
                            rhs=ident[:ln, :ln],
                            is_transpose=True,
                            start=(sub == 0 and i == 0),
                            stop=(sub == 1 and i == 3),
                            skip_group_check=True,
                        )
                if kcp % 4 == 1:
                    nc.scalar.copy(imgT[:, 2 * kcp : 2 * kcp + 2, :], tp)
                else:
                    nc.vector.tensor_copy(imgT[:, 2 * kcp : 2 * kcp + 2, :], tp)
            return imgT

        def ws_tanh_e(g, imgT):
            b0 = 2 * g
            attT = attT_pool.tile([128, AC, L2], BF16, tag="attT")
            for ac in range(AC):
                att = ps_att.tile([128, L2], F32, tag="att")
                for kc in range(KC):
                    nc.tensor.matmul(
                        out=att,
                        lhsT=w_sb[:, kc, ac * 128 : (ac + 1) * 128],
                        rhs=imgT[:, kc, :],
                        start=(kc == 0),
                        stop=(kc == KC - 1),
                    )
                for h in range(2):
                    nc.scalar.activation(
                        out=attT[:, ac, h * L : (h + 1) * L],
                        in_=att[:, h * L : (h + 1) * L],
                        func=AF.Tanh,
                        bias=biasT_sb[:, ac, b0 + h : b0 + h + 1],
                        scale=1.0,
                    )

            e_ps = ps_e.tile([1, L2], F32, tag="e")
            for ac in range(AC):
                nc.tensor.matmul(
                    out=e_ps,
                    lhsT=v_sb[:, ac : ac + 1],
                    rhs=attT[:, ac, :],
                    start=(ac == 0),
                    stop=(ac == AC - 1),
                )
            e_sb = sm.tile([1, L2], F32, tag="e_sb")
            nc.vector.tensor_copy(e_sb, e_ps)
            return e_sb

        def softmax_smalls(g, e_sb):
            """DVE/ACT/DMA only — no PE instructions."""
            b0 = 2 * g
            es2 = sm.tile([2, L], F32, tag="es2")
            nc.sync.dma_start(out=es2, in_=e_sb)
            nm2 = sm.tile([2, 1], F32, tag="nm2")
            nc.vector.tensor_reduce(
                out=nm2, in_=es2, axis=AX.X, op=mybir.AluOpType.max, negate=True
            )
            u2 = sm.tile([2, L], F32, tag="u2")
            s2 = sm.tile([2, 1], F32, tag="s2")
            nc.scalar.activation(
                out=u2, in_=es2, func=AF.Exp, bias=nm2, scale=1.0, accum_out=s2
            )
            r2 = sm.tile([2, 1], F32, tag="r2")
            nc.vector.reciprocal(r2, s2)
            al2 = sm.tile([2, L], F32, tag="al2")
            nc.vector.tensor_scalar_mul(al2, u2, r2)
            nc.sync.dma_start(out=alpha_o[b0 : b0 + 2, :], in_=al2)
            alb2 = sm.tile([2, L], BF16, tag="alb2")
            nc.vector.tensor_copy(alb2, al2)
            return alb2

        def alpha_T(g, alb2):
            alT_ps = ps_alT.tile([128, 4], BF16, tag="alT")
            nc.tensor.matmul(
                out=alT_ps[:, 0:2],
                lhsT=alb2[:, 0:128],
                rhs=ident[:2, :2],
                is_transpose=True,
                start=True,
                stop=False,
                skip_group_check=True,
            )
            nc.tensor.matmul(
                out=alT_ps[: L - 128, 2:4],
                lhsT=alb2[:, 128:L],
                rhs=ident[:2, :2],
                is_transpose=True,
                start=False,
                stop=True,
                skip_group_check=True,
            )
            alT = sm.tile([128, 4], BF16, tag="alT_sb")
            nc.vector.tensor_copy(alT, alT_ps)
            return alT

        def ctx(g, nat, alT):
            b0 = 2 * g
            # batch 0 -> psum partition 0, batch 1 -> partition 32
            ctx_sb = ctxsb_pool.tile([33, E], F32, tag="ctx_sb")
            for n in range(NE):
                ctxp = ps_ctx.tile([33, 512], F32, tag="ctxp")
                for bi in range(2):
                    for lc in range(2):
                        piece = nat[2 * bi + lc]
                        ln = piece.shape[0]
                        lhsT = alT[:ln, 2 * lc + bi : 2 * lc + bi + 1]
                        nc.tensor.matmul(
                            out=ctxp[32 * bi : 32 * bi + 1, :],
                            lhsT=lhsT,
                            rhs=piece[:, n * 512 : (n + 1) * 512],
                            start=(lc == 0),
                            stop=(lc == 1),
                            skip_group_check=True,
                        )
                if n % 2 == 0:
                    nc.vector.tensor_copy(ctx_sb[:, n * 512 : (n + 1) * 512], ctxp)
                else:
                    nc.scalar.copy(ctx_sb[:, n * 512 : (n + 1) * 512], ctxp)
            nc.sync.dma_start(out=ctx_o[b0 : b0 + 2, :], in_=ctx_sb[::32, :])

        # Fine-grained software pipeline. Per-engine queues run in emission
        # order, so: softmax smalls of group g go to DVE/ACT right before
        # group g+1's transposes occupy PE; the tiny alpha-transpose PE ops
        # come after those transposes (softmax long done -> no PE stall),
        # and ctx(g) runs at the end of g+1's front.
        nat = loads(0)
        imgT = transposes(0, nat)
        e_sb = ws_tanh_e(0, imgT)
        for g in range(NG):
            nat_n = loads(g + 1) if g + 1 < NG else None
            alb2 = softmax_smalls(g, e_sb)
            if nat_n is not None:
                imgT_n = transposes(g + 1, nat_n)
                alT = alpha_T(g, alb2)
                e_sb = ws_tanh_e(g + 1, imgT_n)
            else:
                alT = alpha_T(g, alb2)
            ctx(g, nat, alT)
            nat = nat_n


_NC_CACHE = None


def _build():
    global _NC_CACHE
    if _NC_CACHE is None:
        nc = bacc.Bacc("TRN2", target_bir_lowering=False, debug=False)
        with tile.TileContext(nc) as tc:
            _emit(tc)
        nc.compile()
        _NC_CACHE = nc
    return _NC_CACHE


def kernel(img_features, hidden_state, U_w, U_b, W_w, W_b, v_w, v_b):
    img_features = np.asarray(img_features, dtype=np.float32)
    hidden_state = np.asarray(hidden_state, dtype=np.float32)
    U_w = np.asarray(U_w, dtype=np.float32)
    U_b = np.asarray(U_b, dtype=np.float32)
    W_w = np.asarray(W_w, dtype=np.float32)
    W_b = np.asarray(W_b, dtype=np.float32)
    v_w = np.asarray(v_w, dtype=np.float32)

    # host precompute: combined per-(a, b) tanh bias, [A, B]
    biasT = (hidden_state @ U_w).T + (U_b + W_b)[:, None]
    biasT = np.ascontiguousarray(biasT, dtype=np.float32)
    w_bf = W_w.astype(ml_dtypes.bfloat16)
    v_bf = v_w.astype(ml_dtypes.bfloat16)

    nc = _build()
    in_maps = []
    for c in range(NCORES):
        sl = slice(c * BC, (c + 1) * BC)
        in_maps.append(
            {
                "img": np.ascontiguousarray(
                    img_features[sl].reshape(BC * L, E), dtype=np.float32
                ),
                "biasT": np.ascontiguousarray(biasT[:, sl]),
                "w": w_bf,
                "v": v_bf,
            }
        )
    res = run_bass_kernel_spmd(nc, in_maps, list(range(NCORES)))
    ctx = np.concatenate([res.results[c]["ctx"] for c in range(NCORES)], axis=0)
    alpha = np.concatenate([res.results[c]["alpha"] for c in range(NCORES)], axis=0)
    return ctx, alpha
